# revision 19
# baseline (speedup 1.0000x reference)
"""Performer (FAVOR+) attention on 8 Trainium2 NeuronCores.

Sharding: core c handles batch b = c // 2 and head-group g = c % 2
(8 heads of 64 dims each). Host slices/transposes inputs per core, the
Bass kernel computes QKV projection + FAVOR+ for its (b, g) shard, and
the host reassembles the full [4, 4096, 1024] output.

Per-core pipeline (all matmuls bf16, fp32 PSUM accumulation):
  phase 1: qT, kT feature-major [512, 4096]; V token-major [4096, 8*65]
           with a ones column per head (so the context matmul also
           produces ksum = sum_n KP[n,m]).
  phase 2 per head h:
    QP^T = relu(projT_s.T @ qT)      feature-major [256, 4096] (no +eps;
                                      folded in on the host via csum)
    KP   = relu(kT-tiles @ projT_s)  token-major, consumed immediately:
    C_aug^T[f,m] = sum_n V_aug[n,f] KP[n,m]  accumulated over token tiles,
           then corrected with rank-1 terms: +eps*colsum(V_aug) (KP's
           missing +eps) and +b_v*ksum (V projection bias).
    PE-transpose C_aug^T -> C_aug; the C_aug^T copy's accum_out gives
    csum[f] = sum_m C_aug[m,f] for free (host eps correction for QP).
    outT_unnorm [65, 4096] = C_aug^T-weighted QP^T; row 64 = denominator.
  Host: out = (num + eps*csum[:64]) / (den + eps*csum[64]), transpose,
  reassemble.

PSUM tiles are 2-bank [*, 1024] pairs so every elementwise PSUM->SBUF op
moves 1024 columns (per-op overhead dominates DVE/ACT otherwise).

The body repeat count is a runtime input ("reps", uint32 [1,1], loaded
into per-engine registers and used as the For_i trip count). kernel()
always runs reps=1; test.py times the same executable at reps=1 vs
reps=R so the large, kernel-independent axon dispatch floor cancels and
the difference measures true on-device per-inference time (~322us,
within 2% of the TimelineSim cost model's 315us; PE busy is 277us vs a
273us bf16 matmul roofline for this decomposition, i.e. the kernel is
PE-bound at ~88% occupancy). fp8/DoubleRow was evaluated and rejected:
quantizing either QKV-projection operand to e4m3 costs 2.9-4.3e-2
rel_fro against the 2e-2 budget (measured in numpy).
"""

import numpy as np
import ml_dtypes

import concourse.bass as bass
import concourse.tile as tile
from concourse import bacc, mybir
from concourse.bass_utils import run_bass_kernel_spmd
from concourse.masks import make_identity

BF = mybir.dt.bfloat16
F32 = mybir.dt.float32
AF = mybir.ActivationFunctionType
ALU = mybir.AluOpType

HID = 1024
NTOK = 4096
CG = 512          # features per core for each of q/k/v (8 heads * 64)
DH = 64
M = 256           # random-feature dim
EPS = 1e-3
NKT = 8           # 128-row k tiles in HID
HPC = 8           # heads per core
VW = DH + 1       # per-head V width incl. ones column


def _build_kernel(tc, xt, w, bqk, bva, vse, projt, reps_t, outt, csums,
                  static=False):
    nc = tc.nc
    import contextlib
    ctx = contextlib.ExitStack()
    if not static:
        # Runtime-variable repeat count (uint32 [1,1] input). One executable
        # serves both the normal reps=1 path and the timing path (reps=R), so
        # per-executable dispatch-floor differences cancel in differencing.
        regs = nc.alloc_registers("reps_regs", mybir.ALL_ENGINES)
        nc.regs_load(regs, reps_t[0:1, 0:1])
        reps_val = nc.snap(regs, donate=True, min_val=1, max_val=4096)
        loop = ctx.enter_context(tc.For_i(0, reps_val, 1))

    xt_r = xt.rearrange("(kt p) n -> p kt n", p=128)     # [128, 8, 4096]
    w_r = w.rearrange("(kt p) c -> p kt c", p=128)       # [128, 8, 1536]

    res = ctx.enter_context(tc.tile_pool(name="resident", bufs=1))

    def single(shape, dtype, name):
        return res.tile(shape, dtype, name=name, tag=name)

    # ---- resident SBUF tensors ----
    # projT duplicated in both partition halves so heads at base partition 64
    # have a same-base operand (matmul requires matching base partitions)
    proj_sb = single([128, M], BF, "proj_sb")
    bias_sb = single([128, 8], F32, "bias_sb")     # q ct 0-3, k ct 4-7
    bva_sb = single([128, HPC * VW], BF, "bva_sb")   # per-head [b_v | 0]
    vse_sb = single([128, HPC * VW], BF, "vse_sb")   # per-head eps*[vsum | 4096]

    def emit_param_dmas():
        nc.sync.dma_start(out=proj_sb[0:DH, :], in_=projt[:])
        nc.sync.dma_start(out=proj_sb[DH:128, :], in_=projt[:])
        nc.sync.dma_start(out=bias_sb[:, 0:4],
                          in_=bqk[0:CG].rearrange("(c p) -> p c", p=128))
        nc.sync.dma_start(out=bias_sb[:, 4:8],
                          in_=bqk[CG:2 * CG].rearrange("(c p) -> p c", p=128))
        nc.sync.dma_start(out=bva_sb[0:1, :], in_=bva[:])
        nc.sync.dma_start(out=vse_sb[0:1, :], in_=vse[:])

    qk_sb = single([128, 2 * 4 * NTOK], BF, "qk_sb")   # q then k, c-tile major
    v_sb = single([128, 32 * HPC * VW], BF, "v_sb")    # token-major, per-head 65 wide
    csum_sb = single([65, HPC], F32, "csum_sb")

    ones_row = single([128, M], BF, "ones_row")
    nc.vector.memset(ones_row[:], 1.0)
    ident = single([128, 128], BF, "ident")
    make_identity(nc, ident[:])

    # ones columns of V_aug (phase-1 copies only write the d columns; only
    # column DH of each per-head 65-wide slot needs the 1.0 fill)
    nc.gpsimd.memset(
        v_sb[:].rearrange("p (b h e) -> p b h e", h=HPC, e=VW)[:, :, :, DH:DH + 1],
        1.0)

    pbig = ctx.enter_context(tc.tile_pool(name="pbig", bufs=3, space="PSUM"))
    pacc = ctx.enter_context(tc.tile_pool(name="pacc", bufs=1, space="PSUM"))

    # ================= phase 1: QKV projection =================
    with (
        tc.tile_pool(name="wp", bufs=1) as wp,
        tc.tile_pool(name="xp", bufs=2) as xp,
    ):
        w_sb = wp.tile([128, NKT, 3 * CG], BF, tag="w", name="w_sb")
        x_first = xp.tile([128, NKT, 1024], BF, tag="x", name="x_t")
        # w alternates sync/gpsimd queues (the gpsimd queue is free once the
        # ones-column memset shrank); x stays on scalar
        w_engs = [nc.sync, nc.gpsimd]
        for kt in range(NKT):
            w_engs[kt % 2].dma_start(out=w_sb[:, kt, :], in_=w_r[:, kt, :])
            nc.scalar.dma_start(out=x_first[:, kt, :],
                                in_=xt_r[:, kt, 0:1024])
        emit_param_dmas()

        for ntp in range(4):       # 1024-token column pairs
            if ntp == 0:
                x_t = x_first
            else:
                x_t = xp.tile([128, NKT, 1024], BF, tag="x", name="x_t")
                for kt in range(NKT):
                    nc.sync.dma_start(out=x_t[:, kt, :],
                                      in_=xt_r[:, kt, ntp * 1024:(ntp + 1) * 1024])

            # q, k feature-major: psum [c=128, n=1024]
            for qk in range(2):
                for ct in range(4):
                    ps = pbig.tile([128, 1024], F32, tag="pbig", name="qk_ps")
                    coff = qk * CG + ct * 128
                    for half in range(2):
                        for kt in range(NKT):
                            nc.tensor.matmul(
                                ps[:, half * 512:(half + 1) * 512],
                                lhsT=w_sb[:, kt, coff:coff + 128],
                                rhs=x_t[:, kt, half * 512:(half + 1) * 512],
                                start=(kt == 0),
                                stop=(kt == NKT - 1),
                            )
                    off = (qk * 4 + ct) * NTOK + ntp * 1024
                    nc.vector.tensor_scalar_add(
                        qk_sb[:, off:off + 1024], ps[:],
                        bias_sb[:, qk * 4 + ct:qk * 4 + ct + 1])

            # v token-major: psum [n=128, c=512] x2 token blocks per tile
            for stp in range(4):
                ps = pbig.tile([128, 1024], F32, tag="pbig", name="v_ps")
                for half in range(2):
                    st = stp * 2 + half
                    for kt in range(NKT):
                        nc.tensor.matmul(
                            ps[:, half * 512:(half + 1) * 512],
                            lhsT=x_t[:, kt, st * 128:(st + 1) * 128],
                            rhs=w_sb[:, kt, 2 * CG:3 * CG],
                            start=(kt == 0),
                            stop=(kt == NKT - 1),
                        )
                nt1 = ntp * 8 + stp * 2
                dst = v_sb[:, nt1 * HPC * VW:(nt1 + 2) * HPC * VW]
                nc.scalar.activation(
                    dst.rearrange("p (b h e) -> p b h e", b=2, e=VW)[:, :, :, 0:DH],
                    ps[:].rearrange("p (b h d) -> p b h d", b=2, d=DH),
                    AF.Copy,
                )

    # ================= phase 2: FAVOR+ per head =================
    # Software-pipelined: head h's KP/context loop hides head h-1's
    # correction chain, transposes and final output groups in its slots.
    with (
        tc.tile_pool(name="qpp", bufs=2) as qpp,
        tc.tile_pool(name="kpp", bufs=5) as kpp,
        tc.tile_pool(name="cgp", bufs=3) as cgp,
        tc.tile_pool(name="ctp", bufs=2) as ctp,
        tc.tile_pool(name="ksp", bufs=3) as ksp,
        tc.tile_pool(name="otp", bufs=4) as otp,
    ):
        qp_t = {}
        caug_t = {}
        caT_t = {}
        caugT_t = {}
        ksum_t = {}

        def emit_qp(h):
            # QP^T = relu(psum) feature-major [256, 4096] (+eps via host csum)
            hp = (h % 2) * 64
            q_off = (h // 2) * NTOK
            qp_t[h] = qpp.tile([128, 2 * NTOK], BF, tag="qp", name="qp")
            for mt in range(2):
                for ntp in range(4):
                    ps = pbig.tile([128, 1024], F32, tag="pbig", name="qp_ps")
                    for half in range(2):
                        nc.tensor.matmul(
                            ps[:, half * 512:(half + 1) * 512],
                            lhsT=proj_sb[hp:hp + 64, mt * 128:(mt + 1) * 128],
                            rhs=qk_sb[hp:hp + 64,
                                      q_off + ntp * 1024 + half * 512:
                                      q_off + ntp * 1024 + half * 512 + 512],
                            start=True, stop=True,
                        )
                    dst = qp_t[h][:, mt * NTOK + ntp * 1024:mt * NTOK + ntp * 1024 + 1024]
                    if (mt * 4 + ntp) % 2 == 0:
                        nc.scalar.activation(dst, ps[:], AF.Relu)
                    else:
                        nc.vector.tensor_scalar(out=dst, in0=ps[:], scalar1=0.0,
                                                scalar2=None, op0=ALU.max)

        def emit_final_group(h, grp):
            """outT_unnorm [65, 1024] for token cols grp*1024.. of head h."""
            ops = pbig.tile([65, 1024], F32, tag="pbig", name="ops")
            for half in range(2):
                nt = grp * 2 + half
                for mt in range(2):
                    nc.tensor.matmul(
                        ops[:, half * 512:(half + 1) * 512],
                        lhsT=caug_t[h][:, mt, :],
                        rhs=qp_t[h][:, mt * NTOK + nt * 512:mt * NTOK + nt * 512 + 512],
                        start=(mt == 0), stop=(mt == 1),
                    )
            o_t = otp.tile([65, 1024], F32, tag="o", name="o_t")
            if grp % 2 == 0:
                nc.scalar.activation(o_t[:], ops[:], AF.Copy)
            else:
                nc.vector.tensor_copy(o_t[:], ops[:])
            nc.sync.dma_start(
                out=outt[h * 65:(h + 1) * 65, grp * 1024:(grp + 1) * 1024],
                in_=o_t[:])

        def pipe_step(h, kt8):
            """Head h-1 work interleaved into head h's KP loop slot kt8."""
            if h < 0 or (h := h - 1) < 0:
                return
            if kt8 == 0:
                # corrections: +eps*colsum(V_aug) for KP's missing eps
                nc.tensor.matmul(caT_t[h][:], lhsT=vse_sb[0:1, h * VW:(h + 1) * VW],
                                 rhs=ones_row[0:1, :], start=False, stop=False)
                ksum_t[h] = ksp.tile([1, M], BF, tag="ks", name="ksum")
                nc.scalar.activation(ksum_t[h][:], caT_t[h][64:65, :], AF.Copy)
            elif kt8 == 1:
                # +b_v*ksum (V projection bias), then C_aug^T -> sbuf with
                # accum_out = csum (host eps correction for QP)
                nc.tensor.matmul(caT_t[h][:], lhsT=bva_sb[0:1, h * VW:(h + 1) * VW],
                                 rhs=ksum_t[h][:], start=False, stop=True)
                caugT_t[h] = ctp.tile([65, M], BF, tag="caugT", name="caugT")
                nc.scalar.activation(caugT_t[h][:], caT_t[h][:], AF.Copy,
                                     accum_out=csum_sb[:, h:h + 1])
            elif kt8 == 2:
                caug_t[h] = cgp.tile([128, 2, VW], BF, tag="caug", name="caug")
                # inner stride 66 keeps the second transpose 4-byte aligned
                tr = pbig.tile([128, 2, VW + 1], BF, tag="pbig", name="tr")
                for mt in range(2):
                    nc.tensor.transpose(tr[:, mt, 0:VW],
                                        caugT_t[h][:, mt * 128:(mt + 1) * 128],
                                        ident[0:65, 0:65])
                nc.scalar.activation(caug_t[h][:], tr[:, :, 0:VW], AF.Copy)
            elif kt8 <= 6:
                emit_final_group(h, kt8 - 3)
            else:
                if h + 2 < HPC:
                    emit_qp(h + 2)

        emit_qp(0)
        emit_qp(1)
        for h in range(HPC):
            hp = (h % 2) * 64
            k_off = 4 * NTOK + (h // 2) * NTOK

            # KP token-major (4 x 128-token blocks per psum tile) + context
            caT_t[h] = pacc.tile([65, M], F32, tag="acc", name="caT")

            def emit_c_mms(kt8, kp_tile):
                for j in range(4):
                    nt1 = kt8 * 4 + j
                    nc.tensor.matmul(
                        caT_t[h][:],
                        lhsT=v_sb[:, nt1 * HPC * VW + h * VW:
                                  nt1 * HPC * VW + (h + 1) * VW],
                        rhs=kp_tile[:, j * M:(j + 1) * M],
                        start=(kt8 == 0 and j == 0), stop=False)

            # inner software pipeline: C matmuls for tile kt8-1 are emitted
            # between tile kt8's KP matmuls and its relu, so the PE never
            # waits on the relu chain
            prev_kp = None
            for kt8 in range(8):
                kp_ps = pbig.tile([128, 1024], F32, tag="pbig", name="kp_ps")
                for j in range(4):
                    nt1 = kt8 * 4 + j
                    nc.tensor.matmul(
                        kp_ps[:, j * M:(j + 1) * M],
                        lhsT=qk_sb[hp:hp + 64,
                                   k_off + nt1 * 128:k_off + nt1 * 128 + 128],
                        rhs=proj_sb[hp:hp + 64, :],
                        start=True, stop=True,
                    )
                if prev_kp is not None:
                    emit_c_mms(kt8 - 1, prev_kp)
                kp_t = kpp.tile([128, 1024], BF, tag="kp", name="kp_t")
                if kt8 % 2 == 0:
                    nc.vector.tensor_scalar(out=kp_t[:], in0=kp_ps[:],
                                            scalar1=0.0, scalar2=None, op0=ALU.max)
                else:
                    nc.scalar.activation(kp_t[:], kp_ps[:], AF.Relu)
                prev_kp = kp_t
                pipe_step(h, kt8)
            emit_c_mms(7, prev_kp)

        # drain the pipeline for the last head
        for kt8 in range(7):
            pipe_step(HPC, kt8)
            if kt8 == 1:
                nc.sync.dma_start(out=csums[:], in_=csum_sb[:])

    ctx.close()


_NC_CACHE = {}


def _get_nc(static=False):
    key = ("nc", static)
    if key not in _NC_CACHE:
        nc = bacc.Bacc("TRN2", target_bir_lowering=False, debug=False, num_devices=8)
        xt = nc.dram_tensor("xt", [HID, NTOK], BF, kind="ExternalInput").ap()
        w = nc.dram_tensor("w", [HID, 3 * CG], BF, kind="ExternalInput").ap()
        bqk = nc.dram_tensor("bqk", [2 * CG], F32, kind="ExternalInput").ap()
        bva = nc.dram_tensor("bva", [1, HPC * VW], BF, kind="ExternalInput").ap()
        vse = nc.dram_tensor("vse", [1, HPC * VW], BF, kind="ExternalInput").ap()
        projt = nc.dram_tensor("projt", [DH, M], BF, kind="ExternalInput").ap()
        reps_t = nc.dram_tensor("reps", [1, 1], mybir.dt.uint32,
                                kind="ExternalInput").ap()
        outt = nc.dram_tensor("outt", [HPC * VW, NTOK], F32, kind="ExternalOutput").ap()
        csums = nc.dram_tensor("csums", [VW, HPC], F32, kind="ExternalOutput").ap()
        with tile.TileContext(nc) as tc:
            _build_kernel(tc, xt, w, bqk, bva, vse, projt, reps_t, outt, csums,
                          static=static)
        nc.compile()
        _NC_CACHE[key] = nc
    return _NC_CACHE[key]


def _make_in_maps(x, w_qkv, b_qkv, proj, reps=1):
    bf16 = ml_dtypes.bfloat16
    scale = DH ** -0.5
    projt = np.ascontiguousarray((proj.astype(np.float64) * scale).T).astype(bf16)
    in_maps = []
    for c in range(8):
        b, g = divmod(c, 2)
        sl = slice(g * CG, (g + 1) * CG)
        xt = np.ascontiguousarray(x[b].T).astype(bf16)
        wq = w_qkv[:, 0:1024][:, sl].astype(bf16)
        wk = w_qkv[:, 1024:2048][:, sl].astype(bf16)
        wv = w_qkv[:, 2048:3072][:, sl].astype(bf16)
        w = np.ascontiguousarray(np.concatenate([wq, wk, wv], axis=1))
        bqk = np.ascontiguousarray(
            np.concatenate([b_qkv[0:1024][sl], b_qkv[1024:2048][sl]])
        ).astype(np.float32)
        bv = b_qkv[2048:3072][sl].astype(np.float32)
        # per-head strided [b_v | 0] rows for the rank-1 V-bias correction
        bva = np.zeros((1, HPC * VW), np.float32)
        bva.reshape(HPC, VW)[:, 0:DH] = bv.reshape(HPC, DH)
        # eps * colsum(V_aug) per head: vsum from the bf16 operands the
        # device actually uses; ones-column sums to NTOK exactly
        xsum = xt.astype(np.float32).sum(axis=1)           # [1024]
        vsum = xsum @ wv.astype(np.float32) + NTOK * bv    # [512]
        vse = np.zeros((1, HPC * VW), np.float32)
        vse.reshape(HPC, VW)[:, 0:DH] = EPS * vsum.reshape(HPC, DH)
        vse.reshape(HPC, VW)[:, DH] = EPS * NTOK
        in_maps.append({
            "xt": xt, "w": w, "bqk": bqk,
            "bva": bva.astype(bf16), "vse": vse.astype(bf16),
            "projt": projt,
            "reps": np.array([[reps]], dtype=np.uint32),
        })
    return in_maps


def _assemble(results):
    out = np.empty((4, NTOK, HID), np.float32)
    for c in range(8):
        b, g = divmod(c, 2)
        outt = results[c]["outt"]          # [520, 4096]
        csums = results[c]["csums"]        # [65, 8]
        for h in range(HPC):
            num = outt[h * VW:h * VW + DH] + EPS * csums[0:DH, h][:, None]
            den = outt[h * VW + DH] + EPS * csums[DH, h]
            out[b, :, g * CG + h * DH:g * CG + (h + 1) * DH] = (num / den).T
    return out


def run(x, w_qkv, b_qkv, proj, **kwargs):
    nc = _get_nc()
    in_maps = _make_in_maps(x, w_qkv, b_qkv, proj)
    res = run_bass_kernel_spmd(nc, in_maps, list(range(8)), **kwargs)
    return _assemble(res.results), res


def kernel(x, w_qkv, b_qkv, proj):
    x = np.asarray(x)
    w_qkv = np.asarray(w_qkv)
    b_qkv = np.asarray(b_qkv)
    proj = np.asarray(proj)
    out, _ = run(x, w_qkv, b_qkv, proj)
    return out



# revision 21
# speedup vs baseline: 1.0698x; 1.0698x over previous
"""Performer (FAVOR+) attention on 8 Trainium2 NeuronCores.

Sharding: core c handles batch b = c // 2 and head-group g = c % 2
(8 heads of 64 dims each). Host slices/transposes inputs per core, the
Bass kernel computes QKV projection + FAVOR+ for its (b, g) shard, and
the host reassembles the full [4, 4096, 1024] output.

Per-core pipeline (all matmuls bf16, fp32 PSUM accumulation):
  phase 1: qT, kT feature-major [512, 4096]; V token-major [4096, 8*65]
           with a ones column per head (so the context matmul also
           produces ksum = sum_n KP[n,m]).
  phase 2 per head h:
    QP^T = relu(projT_s.T @ qT)      feature-major [256, 4096] (no +eps;
                                      folded in on the host via csum)
    KP   = relu(kT-tiles @ projT_s)  token-major, consumed immediately:
    C_aug^T[f,m] = sum_n V_aug[n,f] KP[n,m]  accumulated over token tiles,
           then corrected with rank-1 terms: +eps*colsum(V_aug) (KP's
           missing +eps) and +b_v*ksum (V projection bias).
    PE-transpose C_aug^T -> C_aug; the C_aug^T copy's accum_out gives
    csum[f] = sum_m C_aug[m,f] for free (host eps correction for QP).
    outT_unnorm [65, 4096] = C_aug^T-weighted QP^T; row 64 = denominator.
  Host: out = (num + eps*csum[:64]) / (den + eps*csum[64]), transpose,
  reassemble.

PSUM tiles are 2-bank [*, 1024] pairs so every elementwise PSUM->SBUF op
moves 1024 columns (per-op overhead dominates DVE/ACT otherwise).

The body repeat count is a runtime input ("reps", uint32 [1,1], loaded
into per-engine registers and used as the For_i trip count). kernel()
always runs reps=1; test.py times the same executable at reps=1 vs
reps=R so the large, kernel-independent axon dispatch floor cancels and
the difference measures true on-device per-inference time (~322us,
within 2% of the TimelineSim cost model's 315us; PE busy is 277us vs a
273us bf16 matmul roofline for this decomposition, i.e. the kernel is
PE-bound at ~88% occupancy). fp8/DoubleRow was evaluated and rejected:
quantizing either QKV-projection operand to e4m3 costs 2.9-4.3e-2
rel_fro against the 2e-2 budget (measured in numpy).
"""

import numpy as np
import ml_dtypes

import concourse.bass as bass
import concourse.tile as tile
from concourse import bacc, mybir
from concourse.bass_utils import run_bass_kernel_spmd
from concourse.masks import make_identity

BF = mybir.dt.bfloat16
F32 = mybir.dt.float32
AF = mybir.ActivationFunctionType
ALU = mybir.AluOpType

HID = 1024
NTOK = 4096
CG = 512          # features per core for each of q/k/v (8 heads * 64)
DH = 64
M = 256           # random-feature dim
EPS = 1e-3
NKT = 8           # 128-row k tiles in HID
HPC = 8           # heads per core
VW = DH + 1       # per-head V width incl. ones column


def _build_kernel(tc, xt, w, bqk, bva, vse, projt, reps_t, outt, csums,
                  static=False):
    nc = tc.nc
    import contextlib
    ctx = contextlib.ExitStack()
    if not static:
        # Runtime-variable repeat count (uint32 [1,1] input). One executable
        # serves both the normal reps=1 path and the timing path (reps=R), so
        # per-executable dispatch-floor differences cancel in differencing.
        regs = nc.alloc_registers("reps_regs", mybir.ALL_ENGINES)
        nc.regs_load(regs, reps_t[0:1, 0:1])
        reps_val = nc.snap(regs, donate=True, min_val=1, max_val=4096)
        loop = ctx.enter_context(tc.For_i(0, reps_val, 1))

    xt_r = xt.rearrange("(kt p) n -> p kt n", p=128)     # [128, 8, 4096]
    w_r = w.rearrange("(kt p) c -> p kt c", p=128)       # [128, 8, 1536]

    res = ctx.enter_context(tc.tile_pool(name="resident", bufs=1))

    def single(shape, dtype, name):
        return res.tile(shape, dtype, name=name, tag=name)

    # ---- resident SBUF tensors ----
    # projT duplicated in both partition halves so heads at base partition 64
    # have a same-base operand (matmul requires matching base partitions)
    proj_sb = single([128, M], BF, "proj_sb")
    bias_sb = single([128, 8], F32, "bias_sb")     # q ct 0-3, k ct 4-7
    bva_sb = single([128, HPC * VW], BF, "bva_sb")   # per-head [b_v | 0]
    vse_sb = single([128, HPC * VW], BF, "vse_sb")   # per-head eps*[vsum | 4096]

    def emit_param_dmas():
        nc.sync.dma_start(out=proj_sb[0:DH, :], in_=projt[:])
        nc.sync.dma_start(out=proj_sb[DH:128, :], in_=projt[:])
        nc.sync.dma_start(out=bias_sb[:, 0:4],
                          in_=bqk[0:CG].rearrange("(c p) -> p c", p=128))
        nc.sync.dma_start(out=bias_sb[:, 4:8],
                          in_=bqk[CG:2 * CG].rearrange("(c p) -> p c", p=128))
        nc.sync.dma_start(out=bva_sb[0:1, :], in_=bva[:])
        nc.sync.dma_start(out=vse_sb[0:1, :], in_=vse[:])

    qk_sb = single([128, 2 * 4 * NTOK], BF, "qk_sb")   # q then k, c-tile major
    v_sb = single([128, 32 * HPC * VW], BF, "v_sb")    # token-major, per-head 65 wide
    csum_sb = single([65, HPC], F32, "csum_sb")

    ones_row = single([128, M], BF, "ones_row")
    nc.vector.memset(ones_row[:], 1.0)
    ident = single([128, 128], BF, "ident")
    make_identity(nc, ident[:])

    # ones columns of V_aug (phase-1 copies only write the d columns)
    nc.gpsimd.memset(v_sb[:], 1.0)

    pbig = ctx.enter_context(tc.tile_pool(name="pbig", bufs=3, space="PSUM"))
    pacc = ctx.enter_context(tc.tile_pool(name="pacc", bufs=1, space="PSUM"))

    # ================= phase 1: QKV projection =================
    with (
        tc.tile_pool(name="wp", bufs=1) as wp,
        tc.tile_pool(name="xp", bufs=2) as xp,
    ):
        w_sb = wp.tile([128, NKT, 3 * CG], BF, tag="w", name="w_sb")
        x_first = xp.tile([128, NKT, 1024], BF, tag="x", name="x_t")
        dma_engs = [nc.sync, nc.scalar]
        for kt in range(NKT):
            dma_engs[0].dma_start(out=w_sb[:, kt, :], in_=w_r[:, kt, :])
            dma_engs[1].dma_start(out=x_first[:, kt, :],
                                   in_=xt_r[:, kt, 0:1024])
        emit_param_dmas()

        for ntp in range(4):       # 1024-token column pairs
            if ntp == 0:
                x_t = x_first
            else:
                x_t = xp.tile([128, NKT, 1024], BF, tag="x", name="x_t")
                for kt in range(NKT):
                    nc.sync.dma_start(out=x_t[:, kt, :],
                                      in_=xt_r[:, kt, ntp * 1024:(ntp + 1) * 1024])

            # q, k feature-major: psum [c=128, n=1024]
            for qk in range(2):
                for ct in range(4):
                    ps = pbig.tile([128, 1024], F32, tag="pbig", name="qk_ps")
                    coff = qk * CG + ct * 128
                    for half in range(2):
                        for kt in range(NKT):
                            nc.tensor.matmul(
                                ps[:, half * 512:(half + 1) * 512],
                                lhsT=w_sb[:, kt, coff:coff + 128],
                                rhs=x_t[:, kt, half * 512:(half + 1) * 512],
                                start=(kt == 0),
                                stop=(kt == NKT - 1),
                            )
                    off = (qk * 4 + ct) * NTOK + ntp * 1024
                    nc.vector.tensor_scalar_add(
                        qk_sb[:, off:off + 1024], ps[:],
                        bias_sb[:, qk * 4 + ct:qk * 4 + ct + 1])

            # v token-major: psum [n=128, c=512] x2 token blocks per tile
            for stp in range(4):
                ps = pbig.tile([128, 1024], F32, tag="pbig", name="v_ps")
                for half in range(2):
                    st = stp * 2 + half
                    for kt in range(NKT):
                        nc.tensor.matmul(
                            ps[:, half * 512:(half + 1) * 512],
                            lhsT=x_t[:, kt, st * 128:(st + 1) * 128],
                            rhs=w_sb[:, kt, 2 * CG:3 * CG],
                            start=(kt == 0),
                            stop=(kt == NKT - 1),
                        )
                nt1 = ntp * 8 + stp * 2
                dst = v_sb[:, nt1 * HPC * VW:(nt1 + 2) * HPC * VW]
                nc.scalar.activation(
                    dst.rearrange("p (b h e) -> p b h e", b=2, e=VW)[:, :, :, 0:DH],
                    ps[:].rearrange("p (b h d) -> p b h d", b=2, d=DH),
                    AF.Copy,
                )

    # ================= phase 2: FAVOR+ per head =================
    # Software-pipelined: head h's KP/context loop hides head h-1's
    # correction chain, transposes and final output groups in its slots.
    with (
        tc.tile_pool(name="qpp", bufs=2) as qpp,
        tc.tile_pool(name="kpp", bufs=5) as kpp,
        tc.tile_pool(name="cgp", bufs=3) as cgp,
        tc.tile_pool(name="ctp", bufs=2) as ctp,
        tc.tile_pool(name="ksp", bufs=3) as ksp,
        tc.tile_pool(name="otp", bufs=4) as otp,
    ):
        qp_t = {}
        caug_t = {}
        caT_t = {}
        caugT_t = {}
        ksum_t = {}

        def emit_qp(h):
            # QP^T = relu(psum) feature-major [256, 4096] (+eps via host csum)
            hp = (h % 2) * 64
            q_off = (h // 2) * NTOK
            qp_t[h] = qpp.tile([128, 2 * NTOK], BF, tag="qp", name="qp")
            for mt in range(2):
                for ntp in range(4):
                    ps = pbig.tile([128, 1024], F32, tag="pbig", name="qp_ps")
                    for half in range(2):
                        nc.tensor.matmul(
                            ps[:, half * 512:(half + 1) * 512],
                            lhsT=proj_sb[hp:hp + 64, mt * 128:(mt + 1) * 128],
                            rhs=qk_sb[hp:hp + 64,
                                      q_off + ntp * 1024 + half * 512:
                                      q_off + ntp * 1024 + half * 512 + 512],
                            start=True, stop=True,
                        )
                    dst = qp_t[h][:, mt * NTOK + ntp * 1024:mt * NTOK + ntp * 1024 + 1024]
                    if (mt * 4 + ntp) % 2 == 0:
                        nc.scalar.activation(dst, ps[:], AF.Relu)
                    else:
                        nc.vector.tensor_scalar(out=dst, in0=ps[:], scalar1=0.0,
                                                scalar2=None, op0=ALU.max)

        def emit_final_group(h, grp):
            """outT_unnorm [65, 1024] for token cols grp*1024.. of head h."""
            ops = pbig.tile([65, 1024], F32, tag="pbig", name="ops")
            for half in range(2):
                nt = grp * 2 + half
                for mt in range(2):
                    nc.tensor.matmul(
                        ops[:, half * 512:(half + 1) * 512],
                        lhsT=caug_t[h][:, mt, :],
                        rhs=qp_t[h][:, mt * NTOK + nt * 512:mt * NTOK + nt * 512 + 512],
                        start=(mt == 0), stop=(mt == 1),
                    )
            o_t = otp.tile([65, 1024], F32, tag="o", name="o_t")
            if grp % 2 == 0:
                nc.scalar.activation(o_t[:], ops[:], AF.Copy)
            else:
                nc.vector.tensor_copy(o_t[:], ops[:])
            nc.sync.dma_start(
                out=outt[h * 65:(h + 1) * 65, grp * 1024:(grp + 1) * 1024],
                in_=o_t[:])

        def pipe_step(h, kt8):
            """Head h-1 work interleaved into head h's KP loop slot kt8."""
            if h < 0 or (h := h - 1) < 0:
                return
            if kt8 == 0:
                # corrections: +eps*colsum(V_aug) for KP's missing eps
                nc.tensor.matmul(caT_t[h][:], lhsT=vse_sb[0:1, h * VW:(h + 1) * VW],
                                 rhs=ones_row[0:1, :], start=False, stop=False)
                ksum_t[h] = ksp.tile([1, M], BF, tag="ks", name="ksum")
                nc.scalar.activation(ksum_t[h][:], caT_t[h][64:65, :], AF.Copy)
            elif kt8 == 1:
                # +b_v*ksum (V projection bias), then C_aug^T -> sbuf with
                # accum_out = csum (host eps correction for QP)
                nc.tensor.matmul(caT_t[h][:], lhsT=bva_sb[0:1, h * VW:(h + 1) * VW],
                                 rhs=ksum_t[h][:], start=False, stop=True)
                caugT_t[h] = ctp.tile([65, M], BF, tag="caugT", name="caugT")
                nc.scalar.activation(caugT_t[h][:], caT_t[h][:], AF.Copy,
                                     accum_out=csum_sb[:, h:h + 1])
            elif kt8 == 2:
                caug_t[h] = cgp.tile([128, 2, VW], BF, tag="caug", name="caug")
                # inner stride 66 keeps the second transpose 4-byte aligned
                tr = pbig.tile([128, 2, VW + 1], BF, tag="pbig", name="tr")
                for mt in range(2):
                    nc.tensor.transpose(tr[:, mt, 0:VW],
                                        caugT_t[h][:, mt * 128:(mt + 1) * 128],
                                        ident[0:65, 0:65])
                nc.scalar.activation(caug_t[h][:], tr[:, :, 0:VW], AF.Copy)
            elif kt8 <= 6:
                emit_final_group(h, kt8 - 3)
            else:
                if h + 2 < HPC:
                    emit_qp(h + 2)

        emit_qp(0)
        emit_qp(1)
        for h in range(HPC):
            hp = (h % 2) * 64
            k_off = 4 * NTOK + (h // 2) * NTOK

            # KP token-major (4 x 128-token blocks per psum tile) + context
            caT_t[h] = pacc.tile([65, M], F32, tag="acc", name="caT")

            def emit_c_mms(kt8, kp_tile):
                for j in range(4):
                    nt1 = kt8 * 4 + j
                    nc.tensor.matmul(
                        caT_t[h][:],
                        lhsT=v_sb[:, nt1 * HPC * VW + h * VW:
                                  nt1 * HPC * VW + (h + 1) * VW],
                        rhs=kp_tile[:, j * M:(j + 1) * M],
                        start=(kt8 == 0 and j == 0), stop=False)

            # inner software pipeline: C matmuls for tile kt8-1 are emitted
            # between tile kt8's KP matmuls and its relu, so the PE never
            # waits on the relu chain
            prev_kp = None
            for kt8 in range(8):
                kp_ps = pbig.tile([128, 1024], F32, tag="pbig", name="kp_ps")
                for j in range(4):
                    nt1 = kt8 * 4 + j
                    nc.tensor.matmul(
                        kp_ps[:, j * M:(j + 1) * M],
                        lhsT=qk_sb[hp:hp + 64,
                                   k_off + nt1 * 128:k_off + nt1 * 128 + 128],
                        rhs=proj_sb[hp:hp + 64, :],
                        start=True, stop=True,
                    )
                if prev_kp is not None:
                    emit_c_mms(kt8 - 1, prev_kp)
                kp_t = kpp.tile([128, 1024], BF, tag="kp", name="kp_t")
                if kt8 % 2 == 0:
                    nc.vector.tensor_scalar(out=kp_t[:], in0=kp_ps[:],
                                            scalar1=0.0, scalar2=None, op0=ALU.max)
                else:
                    nc.scalar.activation(kp_t[:], kp_ps[:], AF.Relu)
                prev_kp = kp_t
                pipe_step(h, kt8)
            emit_c_mms(7, prev_kp)

        # drain the pipeline for the last head
        for kt8 in range(7):
            pipe_step(HPC, kt8)
            if kt8 == 1:
                nc.sync.dma_start(out=csums[:], in_=csum_sb[:])

    ctx.close()


_NC_CACHE = {}


def _get_nc(static=False):
    key = ("nc", static)
    if key not in _NC_CACHE:
        nc = bacc.Bacc("TRN2", target_bir_lowering=False, debug=False, num_devices=8)
        xt = nc.dram_tensor("xt", [HID, NTOK], BF, kind="ExternalInput").ap()
        w = nc.dram_tensor("w", [HID, 3 * CG], BF, kind="ExternalInput").ap()
        bqk = nc.dram_tensor("bqk", [2 * CG], F32, kind="ExternalInput").ap()
        bva = nc.dram_tensor("bva", [1, HPC * VW], BF, kind="ExternalInput").ap()
        vse = nc.dram_tensor("vse", [1, HPC * VW], BF, kind="ExternalInput").ap()
        projt = nc.dram_tensor("projt", [DH, M], BF, kind="ExternalInput").ap()
        reps_t = nc.dram_tensor("reps", [1, 1], mybir.dt.uint32,
                                kind="ExternalInput").ap()
        outt = nc.dram_tensor("outt", [HPC * VW, NTOK], F32, kind="ExternalOutput").ap()
        csums = nc.dram_tensor("csums", [VW, HPC], F32, kind="ExternalOutput").ap()
        with tile.TileContext(nc) as tc:
            _build_kernel(tc, xt, w, bqk, bva, vse, projt, reps_t, outt, csums,
                          static=static)
        nc.compile()
        _NC_CACHE[key] = nc
    return _NC_CACHE[key]


def _make_in_maps(x, w_qkv, b_qkv, proj, reps=1):
    bf16 = ml_dtypes.bfloat16
    scale = DH ** -0.5
    projt = np.ascontiguousarray((proj.astype(np.float64) * scale).T).astype(bf16)
    in_maps = []
    for c in range(8):
        b, g = divmod(c, 2)
        sl = slice(g * CG, (g + 1) * CG)
        xt = np.ascontiguousarray(x[b].T).astype(bf16)
        wq = w_qkv[:, 0:1024][:, sl].astype(bf16)
        wk = w_qkv[:, 1024:2048][:, sl].astype(bf16)
        wv = w_qkv[:, 2048:3072][:, sl].astype(bf16)
        w = np.ascontiguousarray(np.concatenate([wq, wk, wv], axis=1))
        bqk = np.ascontiguousarray(
            np.concatenate([b_qkv[0:1024][sl], b_qkv[1024:2048][sl]])
        ).astype(np.float32)
        bv = b_qkv[2048:3072][sl].astype(np.float32)
        # per-head strided [b_v | 0] rows for the rank-1 V-bias correction
        bva = np.zeros((1, HPC * VW), np.float32)
        bva.reshape(HPC, VW)[:, 0:DH] = bv.reshape(HPC, DH)
        # eps * colsum(V_aug) per head: vsum from the bf16 operands the
        # device actually uses; ones-column sums to NTOK exactly
        xsum = xt.astype(np.float32).sum(axis=1)           # [1024]
        vsum = xsum @ wv.astype(np.float32) + NTOK * bv    # [512]
        vse = np.zeros((1, HPC * VW), np.float32)
        vse.reshape(HPC, VW)[:, 0:DH] = EPS * vsum.reshape(HPC, DH)
        vse.reshape(HPC, VW)[:, DH] = EPS * NTOK
        in_maps.append({
            "xt": xt, "w": w, "bqk": bqk,
            "bva": bva.astype(bf16), "vse": vse.astype(bf16),
            "projt": projt,
            "reps": np.array([[reps]], dtype=np.uint32),
        })
    return in_maps


def _assemble(results):
    out = np.empty((4, NTOK, HID), np.float32)
    for c in range(8):
        b, g = divmod(c, 2)
        outt = results[c]["outt"]          # [520, 4096]
        csums = results[c]["csums"]        # [65, 8]
        for h in range(HPC):
            num = outt[h * VW:h * VW + DH] + EPS * csums[0:DH, h][:, None]
            den = outt[h * VW + DH] + EPS * csums[DH, h]
            out[b, :, g * CG + h * DH:g * CG + (h + 1) * DH] = (num / den).T
    return out


def run(x, w_qkv, b_qkv, proj, **kwargs):
    nc = _get_nc()
    in_maps = _make_in_maps(x, w_qkv, b_qkv, proj)
    res = run_bass_kernel_spmd(nc, in_maps, list(range(8)), **kwargs)
    return _assemble(res.results), res


def kernel(x, w_qkv, b_qkv, proj):
    x = np.asarray(x)
    w_qkv = np.asarray(w_qkv)
    b_qkv = np.asarray(b_qkv)
    proj = np.asarray(proj)
    out, _ = run(x, w_qkv, b_qkv, proj)
    return out

